# revision 3
# baseline (speedup 1.0000x reference)
"""Trainium2 Bass kernel for nn_AR_14328010899741 (v8).

The reference module runs a linear autoregressive scan: starting from the
rolling window buf0 = y.transpose(0,2,1)[:, :, -168:], each of 24 horizon
steps computes pred = buf @ w + b and shifts it into the buffer. Because
every step is linear, the whole scan collapses to

    out[b, h, c] = sum_n A[h, n] * y[b, n, c] + beta[h] * b_scalar

with A [24, 168] / beta [24] computed on the host from (w, b) by running
the same recurrence on basis vectors (float64, ~700k flops). x is unused.

On device this is a memory-bound batched matmul (~12.6 MB HBM traffic per
core at bf16; roofline ~35 us at 358 GB/s + ~10 us fixed framework
pre/postamble). Design:

- y and out staged as bf16 (halves traffic; adds ~3e-3 rel err vs the
  2e-2 gate). Host pre-transposes and pre-splits each core's shard into
  C-halves (y1 [2, 128, 32, 512]) plus the 40-tap tail (y2 [40, 32, 1024])
  so every DMA partition line is contiguous.
- Pipeline quantum is a HALF-iteration (4 batches x 512 channels, 0.5 MB):
  measured, the tail after the last load (final quantum's cold-PE matmul
  wave + DVE + store drain) dominates the gap to roofline, and it scales
  with the quantum.
- Loads ride the sync HWDGE queue in dependency order (t2 before t1
  halves); consts ride scalar; stores ride gpsimd/SWDGE, whose different
  engine-dealing pattern (measured) offsets the 40-line t2's bias toward
  low SDMA-engine slots.
- 4 batches per quantum are packed into the 4 PE column groups via
  tile_position=(0,32j) (M=32 each, A padded with 8 zero columns),
  accumulating K=128+40 into one [128, 512] PSUM bank; one DVE
  tensor_scalar_add per bank adds bias at full 128-lane utilization and
  casts to bf16. Pad rows are stored and stripped on host.
"""

import sys

for _p in ("/opt/trn_rl_repo", "/root/.axon_site", "/root/.axon_site/_ro/trn_rl_repo"):
    if _p not in sys.path:
        sys.path.append(_p)

import numpy as np
import ml_dtypes

B, T, C = 256, 168, 1024
N_SEQ = 168
HORIZON = 24
N_CORES = 8
BPC = B // N_CORES          # batches per core
GRP = 4                     # batches per iteration = PE column groups
K1 = 128                    # first contraction chunk
K2 = N_SEQ - K1             # second contraction chunk (40)
NCHUNK = 512                # matmul moving free dim / PSUM bank / C-half
MPAD = 32                   # padded output rows per column group
NH = C // NCHUNK            # C halves (2)

BF16 = ml_dtypes.bfloat16

_RUNNER = None


def _coeffs(w: np.ndarray, b: np.ndarray):
    """Unroll the AR scan into A [H, N_SEQ] and bias vector [H] (float64)."""
    wv = w[0].astype(np.float64)
    bv = np.float64(b[0])
    coef = np.eye(N_SEQ, dtype=np.float64)      # buffer coeffs wrt initial window
    const = np.zeros(N_SEQ, dtype=np.float64)   # buffer coeffs wrt the bias b
    A = np.zeros((HORIZON, N_SEQ), dtype=np.float64)
    beta = np.zeros(HORIZON, dtype=np.float64)
    for t in range(HORIZON):
        a = wv @ coef
        c = wv @ const + 1.0
        A[t] = a
        beta[t] = c
        coef = np.vstack([coef[1:], a])
        const = np.concatenate([const[1:], [c]])
    return A.astype(np.float32), (beta * bv).astype(np.float32)


def _build():
    import concourse.bass as bass
    import concourse.bacc as bacc
    import concourse.mybir as mybir
    import concourse.tile as tile
    from concourse.bass_utils import run_bass_kernel_spmd

    f32 = mybir.dt.float32
    bf16 = mybir.dt.bfloat16
    f8e4 = mybir.dt.float8e4

    # Bacc (not raw Bass): its generate_event_semaphores pass splits
    # multi-semaphore waits into EventSemaphore instructions, which the
    # single-wait-slot HW instructions require.
    nc = bacc.Bacc("TRN2", target_bir_lowering=False)
    y1_d = nc.dram_tensor("y1", [NH, K1, BPC, NCHUNK], bf16, kind="ExternalInput")
    y2_d = nc.dram_tensor("y2", [K2, BPC, C], f8e4, kind="ExternalInput")
    a1_d = nc.dram_tensor("a1", [K1, MPAD], bf16, kind="ExternalInput")
    a2_d = nc.dram_tensor("a2", [K2, MPAD], bf16, kind="ExternalInput")
    bias_d = nc.dram_tensor("bias", [128, 1], f32, kind="ExternalInput")
    out_d = nc.dram_tensor(
        "out", [BPC // GRP, NH, 128, NCHUNK], bf16, kind="ExternalOutput"
    )

    with tile.TileContext(nc) as tc:
        with (
            tc.tile_pool(name="consts", bufs=1) as consts,
            tc.tile_pool(name="load1", bufs=6) as load1,
            tc.tile_pool(name="load2", bufs=3) as load2,
            tc.tile_pool(name="store", bufs=6) as store,
            tc.tile_pool(name="psum", bufs=6, space="PSUM") as psum,
        ):
            a1 = consts.tile([K1, MPAD], bf16)
            a2 = consts.tile([K2, MPAD], bf16)
            bias = consts.tile([128, 1], f32)
            nc.scalar.dma_start(a1[:], a1_d[:])
            nc.scalar.dma_start(a2[:], a2_d[:])
            nc.scalar.dma_start(bias[:], bias_d[:])

            for i in range(BPC // GRP):
                b0 = i * GRP
                t2 = load2.tile([K2, GRP, C], f8e4, tag="t2")
                nc.sync.dma_start(t2[:], y2_d[:, b0 : b0 + GRP, :])
                for jc in range(NH):
                    cs = slice(jc * NCHUNK, (jc + 1) * NCHUNK)
                    t1 = load1.tile([K1, GRP, NCHUNK], bf16, tag="t1")
                    nc.sync.dma_start(t1[:], y1_d[jc, :, b0 : b0 + GRP, :])
                    osb = store.tile([128, NCHUNK], bf16, tag="osb")
                    ps = psum.tile([128, NCHUNK], f32, tag="ps")
                    for j in range(GRP):
                        nc.tensor.matmul(
                            ps[32 * j : 32 * j + MPAD, :],
                            a1[:],
                            t1[:, j, :],
                            start=True,
                            stop=False,
                            tile_position=(0, 32 * j),
                        )
                    for j in range(GRP):
                        nc.tensor.matmul(
                            ps[32 * j : 32 * j + MPAD, :],
                            a2[:],
                            t2[:, j, cs],
                            start=False,
                            stop=True,
                            tile_position=(0, 32 * j),
                        )
                    nc.vector.tensor_scalar_add(osb[:], ps[:], bias[:])
                    nc.gpsimd.dma_start(out_d[i, jc], osb[:])

    nc.finalize()
    return nc, run_bass_kernel_spmd


def _prep_inputs(y: np.ndarray, w: np.ndarray, b: np.ndarray):
    """Host-side staging: effective weights + per-core transposed bf16 shards."""
    A, bias_vec = _coeffs(np.asarray(w), np.asarray(b))
    # Permute taps so the K2 smallest-|A|-column taps carry the fp8 error:
    # y is staged fp8 for those taps (weights stay bf16; the PE supports
    # mixed bf16-lhsT x fp8-rhs, verified bit-exact on HW). Measured rel
    # err 1.06e-2 vs the 2e-2 gate, for 12% less HBM read traffic.
    order = np.argsort((A.astype(np.float64) ** 2).sum(0))
    perm = np.concatenate([np.sort(order[K2:]), np.sort(order[:K2])])
    At = np.zeros((N_SEQ, MPAD), dtype=np.float32)
    At[:, :HORIZON] = A.T[perm]
    At = At.astype(BF16)
    a1 = np.ascontiguousarray(At[:K1])
    a2 = np.ascontiguousarray(At[K1:])
    bias128 = np.zeros((128, 1), dtype=np.float32)
    for j in range(GRP):
        bias128[32 * j : 32 * j + HORIZON, 0] = bias_vec
    F8 = ml_dtypes.float8_e4m3
    y_f = np.asarray(y, dtype=np.float32)
    in_maps = []
    for c in range(N_CORES):
        shard = y_f[c * BPC : (c + 1) * BPC]                 # [BPC, T, C]
        yt = shard.transpose(1, 0, 2)                        # [T, BPC, C] view
        y1 = np.ascontiguousarray(
            yt[perm[:K1]].reshape(K1, BPC, NH, NCHUNK).transpose(2, 0, 1, 3)
        ).astype(BF16)                                       # [NH, K1, BPC, 512]
        y2 = np.ascontiguousarray(yt[perm[K1:]]).astype(F8)  # [K2, BPC, C]
        in_maps.append(
            {"y1": y1, "y2": y2, "a1": a1, "a2": a2, "bias": bias128}
        )
    return in_maps


def _postprocess(results) -> np.ndarray:
    """[BPC//GRP, NH, 128, 512] bf16 per core -> [B, HORIZON, C] fp32."""
    outs = []
    for r in results:
        o = np.asarray(r["out"])                  # [8, 2, 128, 512] bf16
        o = o.reshape(BPC // GRP, NH, GRP, MPAD, NCHUNK)[:, :, :, :HORIZON, :]
        o = o.transpose(0, 2, 3, 1, 4)            # [8, 4, 24, 2, 512]
        outs.append(o.reshape(BPC, HORIZON, C))
    return np.concatenate(outs, axis=0).astype(np.float32)


def kernel(x: np.ndarray, y: np.ndarray, w: np.ndarray, b: np.ndarray) -> np.ndarray:
    global _RUNNER
    if _RUNNER is None:
        _RUNNER = _build()
    nc, run_spmd = _RUNNER
    in_maps = _prep_inputs(y, w, b)
    res = run_spmd(nc, in_maps, core_ids=list(range(N_CORES)))
    return _postprocess(res.results)
